# revision 7
# baseline (speedup 1.0000x reference)
"""Bass/Trainium2 kernel for the augmented OU Foellmer SDE (STL) sampler.

Strategy: pure data-parallel over batch (8 cores x 2048 rows). On-chip state is
kept TRANSPOSED (yT [128(dim), Bl]) so both MLP matmuls use the tensor engine
with K=dim / K=hidden contractions and large (N=512) moving operands.
Per integration step (all on device):
  hT_j   = tanh(W1_j^T yT + b1_j + t*W1row_j)      4x [128, Bl] (PE + ACT)
  uT     = sum_j W2_j^T hT_j + b2                  (PE accumulate + K=1 bias MM)
  noiseT = PE transpose of natural-layout noise (identity matmul)
  y'     = (1-dt) yT + dt uT + sq noiseT           (scaled-identity matmuls, PSUM)
  raw reductions sum_d(u*noise), sum_d(u*u) via N=1 matmuls against ones column.
Host side: shard/transpose inputs, cumsum + scale the per-step reduction
increments, transpose per-step yT back to natural layout, assemble [21,B,131].
"""

import numpy as np

NSTEPS = 20
DIM = 128
HIDDEN = 512
NCORES = 8
BATCH = 16384

_CACHE = {}
LAST_RESULT = None


def _build(Bl, step_uids, n_uniq):
    import concourse.bass as bass
    import concourse.mybir as mybir
    from concourse.tile import TileContext

    f32 = mybir.dt.float32
    AF = mybir.ActivationFunctionType
    nc = bass.Bass()

    NB = Bl // 128   # b-tiles per core
    NCH = Bl // 512  # 512-wide chunks per core

    yT0 = nc.dram_tensor("yT0", [128, Bl], f32, kind="ExternalInput")
    noises = nc.dram_tensor("noises", [NSTEPS, Bl, 128], f32, kind="ExternalInput")
    W1s = nc.dram_tensor("W1s", [128, 512], f32, kind="ExternalInput")
    W2sb = nc.dram_tensor("W2sb", [128, 512], f32, kind="ExternalInput")
    bias1 = nc.dram_tensor("bias1", [128, NSTEPS * 4], f32, kind="ExternalInput")
    b2row = nc.dram_tensor("b2row", [1, 128], f32, kind="ExternalInput")
    onesrow = nc.dram_tensor("onesrow", [1, 512], f32, kind="ExternalInput")
    onescol = nc.dram_tensor("onescol", [128, 1], f32, kind="ExternalInput")
    idents = nc.dram_tensor(
        "idents", [128, (1 + 3 * n_uniq) * 128], f32, kind="ExternalInput"
    )
    yT_out = nc.dram_tensor("yT_out", [NSTEPS, 128, Bl], f32, kind="ExternalOutput")
    aug_out = nc.dram_tensor("aug_out", [128, NSTEPS * 32], f32, kind="ExternalOutput")

    with TileContext(nc) as tc:
        with (
            tc.tile_pool(name="const", bufs=1) as cpool,
            tc.tile_pool(name="y", bufs=3) as ypool,
            tc.tile_pool(name="nin", bufs=3) as ninpool,
            tc.tile_pool(name="nT", bufs=2) as ntpool,
            tc.tile_pool(name="uT", bufs=2) as utpool,
            tc.tile_pool(name="h", bufs=6) as hpool,
            tc.tile_pool(name="prod", bufs=3) as prodpool,
            tc.tile_pool(name="stage", bufs=1) as stpool,
            tc.tile_pool(name="hps", bufs=2, space="PSUM") as hpsum,
            tc.tile_pool(name="ups", bufs=2, space="PSUM") as upsum,
            tc.tile_pool(name="sps", bufs=2, space="PSUM") as spsum,
        ):
            w1 = cpool.tile([128, 512], f32)
            nc.sync.dma_start(w1, W1s[:, :])
            w2 = cpool.tile([128, 512], f32)
            nc.sync.dma_start(w2, W2sb[:, :])
            b1t = cpool.tile([128, NSTEPS * 4], f32)
            nc.sync.dma_start(b1t, bias1[:, :])
            b2t = cpool.tile([1, 128], f32)
            nc.sync.dma_start(b2t, b2row[:, :])
            onr = cpool.tile([1, 512], f32)
            nc.sync.dma_start(onr, onesrow[:, :])
            onc = cpool.tile([128, 1], f32)
            nc.sync.dma_start(onc, onescol[:, :])
            idt = cpool.tile([128, (1 + 3 * n_uniq) * 128], f32)
            nc.sync.dma_start(idt, idents[:, :])
            aug_stage = stpool.tile([128, NSTEPS * 32], f32)

            def pe_touch(*aps):
                # PE nop that reads the given tiles: consolidates semaphore
                # waits onto the nop so each matmul needs <=1 sync wait
                # (this walrus build rejects multi-wait Matmult lowerings).
                with tc.tile_critical():
                    ins = [nc.tensor.lower_ap(ap) for ap in aps]
                    nop = nc.tensor.nop(hint="dep").ins
                    nop.ins = ins

            ident = idt[:, 0:128]
            y_cur = ypool.tile([128, Bl], f32, tag="y")
            nc.sync.dma_start(y_cur, yT0[:, :])

            for s in range(NSTEPS):
                u = step_uids[s]
                Iom = idt[:, (1 + 3 * u) * 128:(2 + 3 * u) * 128]
                Idt = idt[:, (2 + 3 * u) * 128:(3 + 3 * u) * 128]
                Isq = idt[:, (3 + 3 * u) * 128:(4 + 3 * u) * 128]

                nin = ninpool.tile([128, Bl], f32, tag="nin")
                nc.sync.dma_start(
                    nin.rearrange("p (t d) -> p t d", d=128),
                    noises[s].rearrange("(t p) d -> p t d", p=128),
                )

                # ---- hT_j = tanh(W1_j^T @ yT + bias) ----
                pe_touch(y_cur, w1)
                hts = []
                for j in range(4):
                    ht = hpool.tile([128, Bl], f32, tag="h")
                    hts.append(ht)
                    for half in range(Bl // 1024):
                        hp = hpsum.tile([128, 1024], f32, tag="hps")
                        for q in range(2):
                            o = half * 1024 + q * 512
                            nc.tensor.matmul(
                                hp[:, q * 512:(q + 1) * 512],
                                lhsT=w1[:, j * 128:(j + 1) * 128],
                                rhs=y_cur[:, o:o + 512],
                                start=True, stop=True,
                            )
                        nc.scalar.activation(
                            ht[:, half * 1024:(half + 1) * 1024], hp, AF.Tanh,
                            bias=b1t[:, s * 4 + j:s * 4 + j + 1],
                        )

                # ---- noiseT via PE transpose (matmul against identity) ----
                nT = ntpool.tile([128, Bl], f32, tag="nT")
                pe_touch(nin, idt)
                for c in range(NCH):
                    ntp = spsum.tile([128, 512], f32, tag="sps")
                    for q in range(4):
                        t = c * 4 + q
                        nc.tensor.matmul(
                            ntp[:, q * 128:(q + 1) * 128],
                            lhsT=nin[:, t * 128:(t + 1) * 128],
                            rhs=ident, start=True, stop=True,
                        )
                    nc.vector.tensor_copy(nT[:, c * 512:(c + 1) * 512], ntp)

                # ---- uT = sum_j W2_j^T hT_j + b2 ----
                uT = utpool.tile([128, Bl], f32, tag="uT")
                pe_touch(hts[0], hts[1], hts[2], hts[3], w2, b2t, onr)
                for c in range(NCH):
                    cs = slice(c * 512, (c + 1) * 512)
                    up = upsum.tile([128, 512], f32, tag="ups")
                    for j in range(4):
                        nc.tensor.matmul(
                            up, lhsT=w2[:, j * 128:(j + 1) * 128],
                            rhs=hts[j][:, cs], start=(j == 0), stop=False,
                        )
                    nc.tensor.matmul(up, lhsT=b2t, rhs=onr, start=False, stop=True)
                    nc.vector.tensor_copy(uT[:, cs], up)

                # ---- y' = (1-dt) y + dt u + sq noise ----
                y_new = ypool.tile([128, Bl], f32, tag="y")
                pe_touch(y_cur, uT, nT)
                for c in range(NCH):
                    cs = slice(c * 512, (c + 1) * 512)
                    yp = spsum.tile([128, 512], f32, tag="sps")
                    nc.tensor.matmul(yp, lhsT=Iom, rhs=y_cur[:, cs], start=True, stop=False)
                    nc.tensor.matmul(yp, lhsT=Idt, rhs=uT[:, cs], start=False, stop=False)
                    nc.tensor.matmul(yp, lhsT=Isq, rhs=nT[:, cs], start=False, stop=True)
                    nc.scalar.activation(y_new[:, cs], yp, AF.Copy)
                nc.sync.dma_start(yT_out[s], y_new)

                # ---- raw per-sample reductions over dim (partition axis) ----
                pu = prodpool.tile([128, Bl], f32, tag="prod")
                nc.vector.tensor_mul(pu, uT, nT)
                pe_ = prodpool.tile([128, Bl], f32, tag="prod")
                nc.vector.tensor_mul(pe_, uT, uT)
                ap_ = spsum.tile([128, 512], f32, tag="sps")
                pe_touch(pu, pe_, onc)
                for t in range(NB):
                    nc.tensor.matmul(
                        ap_[:, t:t + 1], lhsT=pu[:, t * 128:(t + 1) * 128],
                        rhs=onc, start=True, stop=True,
                    )
                    nc.tensor.matmul(
                        ap_[:, 16 + t:17 + t], lhsT=pe_[:, t * 128:(t + 1) * 128],
                        rhs=onc, start=True, stop=True,
                    )
                nc.vector.tensor_copy(aug_stage[:, s * 32:(s + 1) * 32], ap_[:, 0:32])

                y_cur = y_new

            nc.sync.dma_start(aug_out[:, :], aug_stage)

    _split_multi_waits(nc, mybir)
    return nc


def _split_multi_waits(nc, mybir):
    """This walrus build allows at most ONE sync-wait per engine instruction
    (except Drain/Barrier).  Hoist excess waits onto single-wait NoOps
    inserted immediately before, on the same engine queue."""
    import copy

    tmpl = None
    for f in nc.m.functions:
        for blk in f.blocks:
            for inst in blk.instructions:
                if isinstance(inst, mybir.InstNoOp) and tmpl is None:
                    tmpl = inst
    assert tmpl is not None, "need at least one native NoOp as template"
    k = 0
    skip = (mybir.InstAllEngineBarrier, mybir.InstEventSemaphore)
    for f in nc.m.functions:
        for blk in f.blocks:
            out = []
            changed = False
            for inst in blk.instructions:
                si = inst.sync_info
                if (
                    si is not None and si.on_wait and len(si.on_wait) > 1
                    and not isinstance(inst, skip)
                ):
                    for w in list(si.on_wait[:-1]):
                        k += 1
                        nop = copy.deepcopy(tmpl)
                        nop.name = f"I-fx{k}"
                        nop.ins = []
                        nop.outs = []
                        nop.engine = inst.engine
                        nop.sync_info = mybir.SyncInfo(on_wait=[w], on_update=[])
                        out.append(nop)
                    inst.sync_info = mybir.SyncInfo(
                        on_wait=[si.on_wait[-1]],
                        on_update=list(si.on_update or []),
                    )
                    changed = True
                out.append(inst)
            if changed:
                blk.instructions = out


def kernel(y0=None, W1=None, b1=None, W2=None, b2=None, noises=None, ts=None, **_):
    global LAST_RESULT
    from concourse.bass_utils import run_bass_kernel_spmd

    y0 = np.asarray(y0, np.float32)
    W1 = np.asarray(W1, np.float32)
    b1 = np.asarray(b1, np.float32)
    W2 = np.asarray(W2, np.float32)
    b2 = np.asarray(b2, np.float32)
    noises = np.asarray(noises, np.float32)
    ts = np.asarray(ts, np.float32)

    B = y0.shape[0]
    nsteps = noises.shape[0]
    assert nsteps == NSTEPS and y0.shape[1] == DIM
    Bl = B // NCORES

    dts = (ts[1:] - ts[:-1]).astype(np.float64)
    sqs = np.sqrt(dts)

    # unique dt values -> scaled identity set
    uniq = []
    step_uids = []
    for s in range(NSTEPS):
        key = round(float(dts[s]), 12)
        if key not in uniq:
            uniq.append(key)
        step_uids.append(uniq.index(key))
    n_uniq = len(uniq)

    ck = (Bl, tuple(step_uids), n_uniq)
    if ck not in _CACHE:
        _CACHE[ck] = _build(Bl, step_uids, n_uniq)
    nc = _CACHE[ck]

    # host-side constant prep
    W1s = np.ascontiguousarray(W1[:128])                      # [128(d), 512]
    W2sb = np.ascontiguousarray(
        W2.reshape(4, 128, 128).transpose(1, 0, 2).reshape(128, 512)
    )                                                         # [128(h'), j*128+d]
    b1m = b1.reshape(4, 128)                                  # [j, p]
    w1last = W1[128].reshape(4, 128)                          # [j, p]
    bias1 = (
        b1m[None, :, :] + ts[:NSTEPS, None, None] * w1last[None, :, :]
    ).transpose(2, 0, 1).reshape(128, NSTEPS * 4)
    bias1 = np.ascontiguousarray(bias1, dtype=np.float32)
    b2row = np.ascontiguousarray(b2[None, :])
    onesrow = np.ones((1, 512), np.float32)
    onescol = np.ones((128, 1), np.float32)
    eye = np.eye(128, dtype=np.float32)
    ids = [eye]
    for d in uniq:
        ids += [np.float32(1.0 - d) * eye, np.float32(d) * eye,
                np.float32(np.sqrt(d)) * eye]
    idents = np.ascontiguousarray(np.concatenate(ids, axis=1))

    in_maps = []
    for m in range(NCORES):
        sl = slice(m * Bl, (m + 1) * Bl)
        in_maps.append({
            "yT0": np.ascontiguousarray(y0[sl].T),
            "noises": np.ascontiguousarray(noises[:, sl, :]),
            "W1s": W1s, "W2sb": W2sb, "bias1": bias1, "b2row": b2row,
            "onesrow": onesrow, "onescol": onescol, "idents": idents,
        })

    res = run_bass_kernel_spmd(nc, in_maps, core_ids=list(range(NCORES)))
    LAST_RESULT = res

    traj = np.zeros((NSTEPS + 1, B, DIM + 3), np.float32)
    traj[0, :, :DIM] = y0
    for m in range(NCORES):
        r = res.results[m]
        sl = slice(m * Bl, (m + 1) * Bl)
        yT = r["yT_out"]                                       # [20, 128, Bl]
        traj[1:, sl, :DIM] = yT.transpose(0, 2, 1)
        aug = r["aug_out"].reshape(128, NSTEPS, 32)            # [p, s, col]
        udw_inc = aug[:, :, :16].transpose(1, 2, 0).reshape(NSTEPS, Bl)
        en_inc = aug[:, :, 16:].transpose(1, 2, 0).reshape(NSTEPS, Bl)
        udw_cum = np.cumsum(udw_inc.astype(np.float64) * sqs[:, None], axis=0)
        en_cum = np.cumsum(en_inc.astype(np.float64) * (0.5 * dts)[:, None], axis=0)
        traj[1:, sl, DIM] = udw_cum
        traj[1:, sl, DIM + 1] = udw_cum
        traj[1:, sl, DIM + 2] = en_cum
    return traj, ts


# revision 9
# speedup vs baseline: 1.2004x; 1.2004x over previous
"""Bass/Trainium2 kernel for the augmented OU Foellmer SDE (STL) sampler.

Strategy: pure data-parallel over batch (8 cores x 2048 rows). On-chip state is
kept TRANSPOSED (yT [128(dim), Bl]) so both MLP matmuls use the tensor engine
with K=dim / K=hidden contractions and large (N=512) moving operands.
Per integration step (all on device):
  hT_j   = tanh(W1_j^T yT + b1_j + t*W1row_j)      4x [128, Bl] (PE + ACT)
  uT     = sum_j W2_j^T hT_j + b2                  (PE accumulate + K=1 bias MM)
  noiseT = PE transpose of natural-layout noise (identity matmul)
  y'     = (1-dt) yT + dt uT + sq noiseT           (scaled-identity matmuls, PSUM)
  raw reductions sum_d(u*noise), sum_d(u*u) via N=1 matmuls against ones column.
Host side: shard/transpose inputs, cumsum + scale the per-step reduction
increments, transpose per-step yT back to natural layout, assemble [21,B,131].
"""

import sys

import numpy as np

for _p in ("/opt/trn_rl_repo", "/root/.axon_site/_ro/trn_rl_repo"):
    if _p not in sys.path:
        sys.path.append(_p)

NSTEPS = 20
DIM = 128
HIDDEN = 512
NCORES = 8
BATCH = 16384

_CACHE = {}
LAST_RESULT = None


def _build(Bl, step_uids, n_uniq):
    import concourse.bass as bass
    import concourse.mybir as mybir
    from concourse.tile import TileContext

    f32 = mybir.dt.float32
    AF = mybir.ActivationFunctionType
    nc = bass.Bass()

    NB = Bl // 128   # b-tiles per core
    NCH = Bl // 512  # 512-wide chunks per core

    yT0 = nc.dram_tensor("yT0", [128, Bl], f32, kind="ExternalInput")
    noises = nc.dram_tensor("noises", [NSTEPS, Bl, 128], f32, kind="ExternalInput")
    W1s = nc.dram_tensor("W1s", [128, 512], f32, kind="ExternalInput")
    W2sb = nc.dram_tensor("W2sb", [128, 512], f32, kind="ExternalInput")
    bias1 = nc.dram_tensor("bias1", [128, NSTEPS * 4], f32, kind="ExternalInput")
    b2row = nc.dram_tensor("b2row", [1, 128], f32, kind="ExternalInput")
    onesrow = nc.dram_tensor("onesrow", [1, 512], f32, kind="ExternalInput")
    onescol = nc.dram_tensor("onescol", [128, 1], f32, kind="ExternalInput")
    idents = nc.dram_tensor(
        "idents", [128, (1 + 3 * n_uniq) * 128], f32, kind="ExternalInput"
    )
    yT_out = nc.dram_tensor("yT_out", [NSTEPS, 128, Bl], f32, kind="ExternalOutput")
    aug_out = nc.dram_tensor("aug_out", [128, NSTEPS * 32], f32, kind="ExternalOutput")

    with TileContext(nc) as tc:
        with (
            tc.tile_pool(name="const", bufs=1) as cpool,
            tc.tile_pool(name="y", bufs=3) as ypool,
            tc.tile_pool(name="nin", bufs=3) as ninpool,
            tc.tile_pool(name="nT", bufs=2) as ntpool,
            tc.tile_pool(name="uT", bufs=2) as utpool,
            tc.tile_pool(name="h", bufs=6) as hpool,
            tc.tile_pool(name="prod", bufs=3) as prodpool,
            tc.tile_pool(name="stage", bufs=1) as stpool,
            tc.tile_pool(name="hps", bufs=2, space="PSUM") as hpsum,
            tc.tile_pool(name="ups", bufs=2, space="PSUM") as upsum,
            tc.tile_pool(name="sps", bufs=2, space="PSUM") as spsum,
        ):
            w1 = cpool.tile([128, 512], f32)
            nc.sync.dma_start(w1, W1s[:, :])
            w2 = cpool.tile([128, 512], f32)
            nc.sync.dma_start(w2, W2sb[:, :])
            b1t = cpool.tile([128, NSTEPS * 4], f32)
            nc.sync.dma_start(b1t, bias1[:, :])
            b2t = cpool.tile([1, 128], f32)
            nc.sync.dma_start(b2t, b2row[:, :])
            onr = cpool.tile([1, 512], f32)
            nc.sync.dma_start(onr, onesrow[:, :])
            onc = cpool.tile([128, 1], f32)
            nc.sync.dma_start(onc, onescol[:, :])
            idt = cpool.tile([128, (1 + 3 * n_uniq) * 128], f32)
            nc.sync.dma_start(idt, idents[:, :])
            aug_stage = stpool.tile([128, NSTEPS * 32], f32)

            def pe_touch(*aps):
                # PE nop that reads the given tiles: consolidates semaphore
                # waits onto the nop so each matmul needs <=1 sync wait
                # (this walrus build rejects multi-wait Matmult lowerings).
                with tc.tile_critical():
                    ins = [nc.tensor.lower_ap(ap) for ap in aps]
                    nop = nc.tensor.nop(hint="dep").ins
                    nop.ins = ins

            ident = idt[:, 0:128]
            pe_touch(idt)  # emits one native NoOp: template for _split_multi_waits
            y_cur = ypool.tile([128, Bl], f32, tag="y")
            nc.sync.dma_start(y_cur, yT0[:, :])

            for s in range(NSTEPS):
                u = step_uids[s]
                Iom = idt[:, (1 + 3 * u) * 128:(2 + 3 * u) * 128]
                Idt = idt[:, (2 + 3 * u) * 128:(3 + 3 * u) * 128]
                Isq = idt[:, (3 + 3 * u) * 128:(4 + 3 * u) * 128]

                nin = ninpool.tile([128, Bl], f32, tag="nin")
                nc.sync.dma_start(
                    nin.rearrange("p (t d) -> p t d", d=128),
                    noises[s].rearrange("(t p) d -> p t d", p=128),
                )

                # ---- hT_j = tanh(W1_j^T @ yT + bias) ----
                hts = []
                for j in range(4):
                    ht = hpool.tile([128, Bl], f32, tag="h")
                    hts.append(ht)
                    for half in range(Bl // 1024):
                        hp = hpsum.tile([128, 1024], f32, tag="hps")
                        for q in range(2):
                            o = half * 1024 + q * 512
                            nc.tensor.matmul(
                                hp[:, q * 512:(q + 1) * 512],
                                lhsT=w1[:, j * 128:(j + 1) * 128],
                                rhs=y_cur[:, o:o + 512],
                                start=True, stop=True,
                            )
                        nc.scalar.activation(
                            ht[:, half * 1024:(half + 1) * 1024], hp, AF.Tanh,
                            bias=b1t[:, s * 4 + j:s * 4 + j + 1],
                        )

                # ---- noiseT via PE transpose (matmul against identity) ----
                nT = ntpool.tile([128, Bl], f32, tag="nT")
                for c in range(NCH):
                    ntp = spsum.tile([128, 512], f32, tag="sps")
                    for q in range(4):
                        t = c * 4 + q
                        nc.tensor.matmul(
                            ntp[:, q * 128:(q + 1) * 128],
                            lhsT=nin[:, t * 128:(t + 1) * 128],
                            rhs=ident, start=True, stop=True,
                        )
                    nc.vector.tensor_copy(nT[:, c * 512:(c + 1) * 512], ntp)

                # ---- uT = sum_j W2_j^T hT_j + b2 ----
                uT = utpool.tile([128, Bl], f32, tag="uT")
                for c in range(NCH):
                    cs = slice(c * 512, (c + 1) * 512)
                    up = upsum.tile([128, 512], f32, tag="ups")
                    for j in range(4):
                        nc.tensor.matmul(
                            up, lhsT=w2[:, j * 128:(j + 1) * 128],
                            rhs=hts[j][:, cs], start=(j == 0), stop=False,
                        )
                    nc.tensor.matmul(up, lhsT=b2t, rhs=onr, start=False, stop=True)
                    nc.vector.tensor_copy(uT[:, cs], up)

                # ---- y' = (1-dt) y + dt u + sq noise ----
                y_new = ypool.tile([128, Bl], f32, tag="y")
                for c in range(NCH):
                    cs = slice(c * 512, (c + 1) * 512)
                    yp = spsum.tile([128, 512], f32, tag="sps")
                    nc.tensor.matmul(yp, lhsT=Iom, rhs=y_cur[:, cs], start=True, stop=False)
                    nc.tensor.matmul(yp, lhsT=Idt, rhs=uT[:, cs], start=False, stop=False)
                    nc.tensor.matmul(yp, lhsT=Isq, rhs=nT[:, cs], start=False, stop=True)
                    nc.scalar.activation(y_new[:, cs], yp, AF.Copy)
                nc.sync.dma_start(yT_out[s], y_new)

                # ---- raw per-sample reductions over dim (partition axis) ----
                pu = prodpool.tile([128, Bl], f32, tag="prod")
                nc.vector.tensor_mul(pu, uT, nT)
                pe_ = prodpool.tile([128, Bl], f32, tag="prod")
                nc.vector.tensor_mul(pe_, uT, uT)
                ap_ = upsum.tile([128, 512], f32, tag="ups")
                for t in range(NB):
                    nc.tensor.matmul(
                        ap_[:, t:t + 1], lhsT=pu[:, t * 128:(t + 1) * 128],
                        rhs=onc, start=True, stop=True,
                    )
                    nc.tensor.matmul(
                        ap_[:, 16 + t:17 + t], lhsT=pe_[:, t * 128:(t + 1) * 128],
                        rhs=onc, start=True, stop=True,
                    )
                nc.vector.tensor_copy(aug_stage[:, s * 32:(s + 1) * 32], ap_[:, 0:32])

                y_cur = y_new

            nc.sync.dma_start(aug_out[:, :], aug_stage)

    _split_multi_waits(nc, mybir)
    return nc


def _split_multi_waits(nc, mybir):
    """This walrus build allows at most ONE sync-wait per engine instruction
    (except Drain/Barrier).  Hoist excess waits onto single-wait NoOps
    inserted immediately before, on the same engine queue."""
    import copy

    tmpl = None
    for f in nc.m.functions:
        for blk in f.blocks:
            for inst in blk.instructions:
                if isinstance(inst, mybir.InstNoOp) and tmpl is None:
                    tmpl = inst
    assert tmpl is not None, "need at least one native NoOp as template"
    k = 0
    skip = (mybir.InstAllEngineBarrier, mybir.InstEventSemaphore)
    for f in nc.m.functions:
        for blk in f.blocks:
            out = []
            changed = False
            for inst in blk.instructions:
                si = inst.sync_info
                if (
                    si is not None and si.on_wait and len(si.on_wait) > 1
                    and not isinstance(inst, skip)
                ):
                    for w in list(si.on_wait[:-1]):
                        k += 1
                        nop = copy.deepcopy(tmpl)
                        nop.name = f"I-fx{k}"
                        nop.ins = []
                        nop.outs = []
                        nop.engine = inst.engine
                        nop.sync_info = mybir.SyncInfo(on_wait=[w], on_update=[])
                        out.append(nop)
                    inst.sync_info = mybir.SyncInfo(
                        on_wait=[si.on_wait[-1]],
                        on_update=list(si.on_update or []),
                    )
                    changed = True
                out.append(inst)
            if changed:
                blk.instructions = out


def kernel(y0=None, W1=None, b1=None, W2=None, b2=None, noises=None, ts=None, **_):
    global LAST_RESULT
    from concourse.bass_utils import run_bass_kernel_spmd

    y0 = np.asarray(y0, np.float32)
    W1 = np.asarray(W1, np.float32)
    b1 = np.asarray(b1, np.float32)
    W2 = np.asarray(W2, np.float32)
    b2 = np.asarray(b2, np.float32)
    noises = np.asarray(noises, np.float32)
    ts = np.asarray(ts, np.float32)

    B = y0.shape[0]
    nsteps = noises.shape[0]
    assert nsteps == NSTEPS and y0.shape[1] == DIM
    Bl = B // NCORES

    dts = (ts[1:] - ts[:-1]).astype(np.float64)
    sqs = np.sqrt(dts)

    # unique dt values -> scaled identity set
    uniq = []
    step_uids = []
    for s in range(NSTEPS):
        key = round(float(dts[s]), 12)
        if key not in uniq:
            uniq.append(key)
        step_uids.append(uniq.index(key))
    n_uniq = len(uniq)

    ck = (Bl, tuple(step_uids), n_uniq)
    if ck not in _CACHE:
        _CACHE[ck] = _build(Bl, step_uids, n_uniq)
    nc = _CACHE[ck]

    # host-side constant prep
    W1s = np.ascontiguousarray(W1[:128])                      # [128(d), 512]
    W2sb = np.ascontiguousarray(
        W2.reshape(4, 128, 128).transpose(1, 0, 2).reshape(128, 512)
    )                                                         # [128(h'), j*128+d]
    b1m = b1.reshape(4, 128)                                  # [j, p]
    w1last = W1[128].reshape(4, 128)                          # [j, p]
    bias1 = (
        b1m[None, :, :] + ts[:NSTEPS, None, None] * w1last[None, :, :]
    ).transpose(2, 0, 1).reshape(128, NSTEPS * 4)
    bias1 = np.ascontiguousarray(bias1, dtype=np.float32)
    b2row = np.ascontiguousarray(b2[None, :])
    onesrow = np.ones((1, 512), np.float32)
    onescol = np.ones((128, 1), np.float32)
    eye = np.eye(128, dtype=np.float32)
    ids = [eye]
    for d in uniq:
        ids += [np.float32(1.0 - d) * eye, np.float32(d) * eye,
                np.float32(np.sqrt(d)) * eye]
    idents = np.ascontiguousarray(np.concatenate(ids, axis=1))

    in_maps = []
    for m in range(NCORES):
        sl = slice(m * Bl, (m + 1) * Bl)
        in_maps.append({
            "yT0": np.ascontiguousarray(y0[sl].T),
            "noises": np.ascontiguousarray(noises[:, sl, :]),
            "W1s": W1s, "W2sb": W2sb, "bias1": bias1, "b2row": b2row,
            "onesrow": onesrow, "onescol": onescol, "idents": idents,
        })

    res = run_bass_kernel_spmd(nc, in_maps, core_ids=list(range(NCORES)))
    LAST_RESULT = res

    traj = np.zeros((NSTEPS + 1, B, DIM + 3), np.float32)
    traj[0, :, :DIM] = y0
    for m in range(NCORES):
        r = res.results[m]
        sl = slice(m * Bl, (m + 1) * Bl)
        yT = r["yT_out"]                                       # [20, 128, Bl]
        traj[1:, sl, :DIM] = yT.transpose(0, 2, 1)
        aug = r["aug_out"].reshape(128, NSTEPS, 32)            # [p, s, col]
        udw_inc = aug[:, :, :16].transpose(1, 2, 0).reshape(NSTEPS, Bl)
        en_inc = aug[:, :, 16:].transpose(1, 2, 0).reshape(NSTEPS, Bl)
        udw_cum = np.cumsum(udw_inc.astype(np.float64) * sqs[:, None], axis=0)
        en_cum = np.cumsum(en_inc.astype(np.float64) * (0.5 * dts)[:, None], axis=0)
        traj[1:, sl, DIM] = udw_cum
        traj[1:, sl, DIM + 1] = udw_cum
        traj[1:, sl, DIM + 2] = en_cum
    return traj, ts
